# revision 1
# baseline (speedup 1.0000x reference)
"""Dynamic depthwise 3x3 conv (per-pixel weights) on 8 Trainium2 NeuronCores.

Problem:
  x:            [4, 64, 256, 256]  f32
  conv_weights: [4, 576, 256, 256] f32  (= [4, 64ch * 9tap, 256, 256])
  out[n,c,h,w] = sum_k w[n, c*9+k, h, w] * xpad[n, c, h+ki, w+kj],  k=(ki,kj) row-major

Sharding: pure data parallel over (batch n, H-half) -> 8 shards.

On-core layout: partition p = hb*64 + c (hb in {0,1} picks a 64-row block of
the core's 128 output rows, c the channel). x is stored UNPADDED in W
(rows of 256, H-padded on the host), so the flattened (h, w) index j is
contiguous and a single custom-DVE segmented-MAC instruction covers a whole
Rh-row tile for one kernel-row dh:

    tgt[p, j] = sum_dw w[p, dh, j, dw] * x[p, j + dh*256 + dw - 1]

Width-edge taps (wd=0,dw=0 and wd=255,dw=2) would wrap into the neighbouring
row; in the reference those taps multiply zero-padding, so the host repack
zeroes those weight entries and the wrap contributions vanish exactly.

conv_weights are repacked host-side to [T, 128, dh, (h,w), dw] so each
DMA is one sequential HBM stream and the MAC's dw-segments are innermost.
The custom DVE op does the 3-tap dot product per output element in one
pass with a per-segment scan reset.

Memory-regime optimizations (HBM cap ~358 GB/s/NC, 16 SDMA engines):
 - everything device-side is bf16 (host casts f32->bf16, upcasts y back);
   halves the dominant 72MB/core weight traffic; rel err ~4e-3 << 2e-2.
 - the three dh partial planes are combined with SBUF->SBUF accumulating
   DMAs (gpsimd SWDGE, accum_op=add) instead of DVE adds: the DVE then
   runs only the 24 segmacs back-to-back (~157us, the critical path).
   The combine DMAs trail by DEFER=2 tiles so they don't crowd the w
   prefetch stream at the moment it is needed.
 - startup: only the first Rh+2 x rows + the first dh chunk of w tile 0
   gate the first segmac; remaining x tiles load staggered at t=1,3,5.
"""

import sys

sys.path.insert(0, "/opt/trn_rl_repo")

import ml_dtypes
import numpy as np

import concourse.bass as bass
import concourse.bacc as bacc
import concourse.tile as tile
from concourse import mybir
from concourse.bass_utils import run_bass_kernel_spmd


# ---------------------------------------------------------------------------
# Custom DVE op: segmented multiply-accumulate (dot-KW per output element).
#   out[p, s] = sum_n in0[p, s, n] * in1[p, s, n]
# A scan(ADD, Src0*Src1) whose accumulator resets at each SUB_DIM_DONE (the
# per-page "per_subdim" STEP state the HW supports but the stock Spec DSL does
# not expose); the out AP uses a step-0 inner dim so the last (complete)
# partial of each segment is what lands at out[p, s]. Streams both tensors at
# 1 elem/lane/cycle: a 3-tap dot product costs 3 input cycles, no reduction
# passes.
# ---------------------------------------------------------------------------

from dataclasses import dataclass

import concourse.dve_spec as dve_spec
import concourse.dve_ops as dve_ops
from concourse.dve_spec import AluOp, Spec, Src0, Src1
from concourse.dve_uop import DveOpSpec

OP_NAME = "SEG_MAC_ANT"



@dataclass(frozen=True)
class _ResetScan(dve_spec.Scan):
    """scan() that re-seeds from `init` at each SUB_DIM_DONE."""


def _patched_scan_overrides(scans, node_stage):
    seed, step = {}, {}
    for scan in scans:
        d = node_stage[scan]
        init = dve_spec._scan_init(scan)
        seed[d] = dve_spec._node_as_stage(init)
        if isinstance(scan, _ResetScan):
            # Page boundary: restart the fold — d = init op expr (the
            # "per_subdim" STEP variant from the HW state-machine table).
            step[d] = dve_spec._Stage(scan.op, init, scan.expr)
        elif scan._subdim_step is not None:
            step[d] = dve_spec._Stage(
                scan.op, dve_spec.AluInp.CURR_ALU_OUT, scan._subdim_step
            )
    return seed, step


def _segmac_ref(in0, in1, c0, c1, c2):
    # CoreSim reference: per-segment inclusive prefix of the products.
    return np.cumsum(
        np.asarray(in0, np.float32) * np.asarray(in1, np.float32),
        axis=-1,
        dtype=np.float32,
    )


def get_segmac_op():
    """Build + register the op (idempotent). Returns the DveOp."""
    existing = getattr(dve_ops, "_ANT_SEG_MAC", None)
    if existing is not None:
        return existing

    dve_spec._scan_overrides = _patched_scan_overrides

    body = _ResetScan(AluOp.ADD, Src0 * Src1)
    spec = Spec(body=body, reference=_segmac_ref)

    shas = {}
    for ver in ("v3", "v4"):
        uops = dve_spec.lower(spec, ver=ver)
        shas[ver] = DveOpSpec(name=OP_NAME, uops=uops, rd1_en=True).sha(ver)

    op = dve_ops.DveOp(OP_NAME, spec, subdim=True, uops_sha=shas)
    dve_ops.OPS.append(op)
    dve_ops._SUB_OPCODE_FOR_NAME[OP_NAME] = (
        dve_ops._CUSTOM_DVE_ROW_BASE + len(dve_ops.OPS) - 1
    )
    dve_ops.CUSTOM_DVE_SPECS[OP_NAME] = spec
    assert dve_ops._SUB_OPCODE_FOR_NAME[OP_NAME] < 0x20
    dve_ops._ANT_SEG_MAC = op
    return op


def window_ap(sl, dims):
    """Build an AP over `sl`'s tensor/offset with explicit free dims
    [[step, count], ...] (partition dim copied from sl)."""
    import bass_rust

    return bass_rust.AP(
        sl.tensor,
        sl.offset,
        [list(sl.ap[0])] + [list(d) for d in dims],
        sl.const_val,
        sl.runtime_checks,
        sl.dep_tracking_offset,
    )


N, C, H, W = 4, 64, 256, 256
KW = 3
NCORES = 8
HH = H // 2          # rows per core
RB = HH // 2         # rows per partition block (64)
Rh = 8               # rows per h-tile
T = RB // Rh         # h-tiles per core
NXT = 4              # resident x tiles per core
XB = RB // NXT       # local output rows covered per x tile (16)
XR = XB + 2          # rows per resident x tile incl halo
XF = XR * W + 2      # x tile free elems incl 1 zero guard at each end
J = Rh * W           # flattened (h, w) positions per tile
WF = KW * KW * J     # w tile free elems
F32 = mybir.dt.float32
BF16 = mybir.dt.bfloat16
NPBF16 = ml_dtypes.bfloat16

_CACHE = {}


def _build():
    op = get_segmac_op()
    nc = bacc.Bacc("TRN2", target_bir_lowering=False, debug=False, num_devices=NCORES)
    x_in = nc.dram_tensor("x", [NXT, 128, XF], BF16, kind="ExternalInput")
    w_in = nc.dram_tensor("w", [T, 128, WF], BF16, kind="ExternalInput")
    y_out = nc.dram_tensor("y", [T, 128, J], BF16, kind="ExternalOutput")

    with tile.TileContext(nc) as tc:
        with (
            tc.tile_pool(name="xp", bufs=1) as xpool,
            tc.tile_pool(name="wp", bufs=3) as wpool,
            tc.tile_pool(name="op", bufs=4) as opool,
            tc.tile_pool(name="pa", bufs=3) as papool,
            tc.tile_pool(name="pb", bufs=3) as pbpool,
        ):
            # x stays resident: NXT tiles, each covering XB output rows
            # (+2 halo rows) per partition block, loaded once. x0 is issued
            # first (tile-0 compute needs it); x1-3 follow behind w0 so the
            # first segmac isn't queued behind 4.7MB of x.
            xtiles = []
            for s in range(NXT):
                xt = xpool.tile([128, XF], BF16, tag=f"x{s}")
                xtiles.append(xt)
            # only the first Rh+2 rows of x0 gate tile 0's compute
            XC0 = (Rh + 2) * W + 2
            nc.scalar.dma_start(
                out=xtiles[0][:, 0:XC0], in_=x_in[0, :, 0:XC0]
            )

            DEFER = 2  # trail the combine DMAs so they don't crowd w prefetch
            ots, pas, pbs = {}, {}, {}

            def combine(u):
                # SBUF->SBUF accumulating DMAs (SWDGE CCE add) do the
                # plane combine off the DVE: only descriptor work for
                # GpSimd, transfer rides idle SDMA/fabric capacity.
                # The final tile instead combines on the (then idle) DVE —
                # two bf16-2x adds beat the serialized accum-DMA chain on
                # the drain tail.
                ot, pa, pb = ots.pop(u), pas.pop(u), pbs.pop(u)
                if u == T - 1:
                    nc.vector.tensor_add(ot[:], ot[:], pa[:])
                    nc.vector.tensor_add(ot[:], ot[:], pb[:])
                else:
                    nc.gpsimd.dma_start(
                        out=ot[:], in_=pa[:], accum_op=mybir.AluOpType.add
                    )
                    nc.gpsimd.dma_start(
                        out=ot[:], in_=pb[:], accum_op=mybir.AluOpType.add
                    )
                nc.scalar.dma_start(out=y_out[u], in_=ot[:])

            for t in range(T):
                wt = wpool.tile([128, WF], BF16)
                # every w tile loads as 3 per-dh chunks: 12KB packets run at
                # the same per-engine rate as whole-tile, and subtile deps
                # let each segmac start as soon as its own third lands.
                for dh in range(KW):
                    c0 = dh * KW * J
                    nc.sync.dma_start(
                        out=wt[:, c0:c0 + KW * J],
                        in_=w_in[t, :, c0:c0 + KW * J],
                    )
                if t == 0:
                    # backfill the rest of x0 behind w0's chunks
                    nc.scalar.dma_start(
                        out=xtiles[0][:, XC0:XF], in_=x_in[0, :, XC0:XF]
                    )
                if t in (1, 3, 5):
                    # stagger the remaining resident-x loads so they don't
                    # compete with the early w tiles (x_s first needed at
                    # tile 2s).
                    s = (t + 1) // 2
                    nc.scalar.dma_start(out=xtiles[s][:], in_=x_in[s])

                xt = xtiles[t * Rh // XB]
                rbase = t * Rh - (t * Rh // XB) * XB

                ot = ots[t] = opool.tile([128, J], BF16, name="ot")
                pa = pas[t] = papool.tile([128, J], BF16, name="pa")
                pb = pbs[t] = pbpool.tile([128, J], BF16, name="pb")
                # one whole-tile segmented MAC per kernel row dh:
                #   tgt[p, j] = sum_dw w[dh, j, dw] * x[(rbase+dh)*W + j + dw - 1]
                # (x AP offset: the +1 guard shift and -1 dw base cancel)
                for dh, tgt in ((0, ot), (1, pa), (2, pb)):
                    w_sl = wt[:, dh * KW * J:(dh + 1) * KW * J]
                    x_sl = xt[:, (rbase + dh) * W:(rbase + dh) * W + J + 2]
                    nc.vector._custom_dve(
                        op,
                        out=window_ap(tgt[:, 0:J], [[1, J], [0, KW]]),
                        in0=window_ap(w_sl, [[KW, J], [1, KW]]),
                        in1=window_ap(x_sl, [[1, J], [1, KW]]),
                    )
                if t >= DEFER:
                    combine(t - DEFER)
            for u in range(max(0, T - DEFER), T):
                combine(u)
    nc.compile()
    return nc


def _get_nc():
    if "nc" not in _CACHE:
        _CACHE["nc"] = _build()
    return _CACHE["nc"]


def _pack_core(xh_n: np.ndarray, w5_n: np.ndarray, hf: int):
    """Repack one core's shard into per-tile-contiguous DMA blocks.

    xh_n: [C, H+2, W] H-padded x for batch n; w5_n: [C, 9, H, W].
    Returns x_blocks [NXT, 128, XF], w_blocks [T, 128, WF].
    """
    xc = xh_n[:, hf * HH:hf * HH + HH + 2, :]          # [C, HH+2, W]
    wc = w5_n[:, :, hf * HH:(hf + 1) * HH, :]          # [C, 9, HH, W]

    xb = np.zeros((NXT, 2, C, XR * W + 2), dtype=np.float32)
    for s in range(NXT):
        for hb in range(2):
            r0 = hb * RB + s * XB
            xb[s, hb, :, 1:-1] = xc[:, r0:r0 + XR, :].reshape(C, XR * W)
    # w: [C, (dh, dw), (hb, t, r), wd] -> [t, (hb, c), dh, (r, wd), dw]
    wb = (
        wc.reshape(C, KW, KW, 2, T, Rh, W)
        .transpose(4, 3, 0, 1, 5, 6, 2)
        .copy()
    )  # [T, hb, C, dh, r, wd, dw]
    # width-edge taps multiply zero padding in the reference -> zero them
    wb[:, :, :, :, :, 0, 0] = 0.0
    wb[:, :, :, :, :, W - 1, KW - 1] = 0.0
    return (
        xb.reshape(NXT, 128, XF).astype(NPBF16),
        np.ascontiguousarray(wb.reshape(T, 128, WF)).astype(NPBF16),
    )


def _make_in_maps(x: np.ndarray, conv_weights: np.ndarray):
    x = np.asarray(x, dtype=np.float32)
    w5 = np.asarray(conv_weights, dtype=np.float32).reshape(N, C, KW * KW, H, W)
    xh = np.pad(x, ((0, 0), (0, 0), (1, 1), (0, 0)))

    in_maps = []
    for i in range(NCORES):
        n, hf = divmod(i, 2)
        xb, wb = _pack_core(xh[n], w5[n], hf)
        in_maps.append({"x": xb, "w": wb})
    return in_maps


def kernel(x: np.ndarray, conv_weights: np.ndarray) -> np.ndarray:
    nc = _get_nc()
    in_maps = _make_in_maps(x, conv_weights)
    res = run_bass_kernel_spmd(nc, in_maps, list(range(NCORES)))
    out = np.empty((N, C, H, W), dtype=np.float32)
    for i in range(NCORES):
        n, hf = divmod(i, 2)
        yb = res.results[i]["y"].astype(np.float32).reshape(T, 2, C, Rh, W)
        # invert: out rows h = hf*HH + hb*RB + t*Rh + h_sub
        oc = yb.transpose(2, 1, 0, 3, 4).reshape(C, HH, W)
        out[n, :, hf * HH:(hf + 1) * HH, :] = oc
    return out



# revision 2
# speedup vs baseline: 1.4994x; 1.4994x over previous
"""Dynamic depthwise 3x3 conv (per-pixel weights) on 8 Trainium2 NeuronCores.

Problem:
  x:            [4, 64, 256, 256]  f32
  conv_weights: [4, 576, 256, 256] f32  (= [4, 64ch * 9tap, 256, 256])
  out[n,c,h,w] = sum_k w[n, c*9+k, h, w] * xpad[n, c, h+ki, w+kj],  k=(ki,kj) row-major

Sharding: pure data parallel over (batch n, H-half) -> 8 shards.

On-core layout: partition p = hb*64 + c (hb in {0,1} picks a 64-row block of
the core's 128 output rows, c the channel). x is stored UNPADDED in W
(rows of 256, H-padded on the host), so the flattened (h, w) index j is
contiguous; per kernel-row dh one custom-DVE FIR instruction computes a whole
Rh-row tile:

    tgt[p, j] = sum_dw w[p, dh, j, dw] * x[p, j + dh*256 + dw - 1]

Width-edge taps (wd=0,dw=0 and wd=255,dw=2) would wrap into the neighbouring
row; in the reference those taps multiply zero-padding, so the host repack
zeroes those weight entries and the wrap contributions vanish exactly.

conv_weights are repacked host-side to [T, 128, dh, (h,w), dw] so each
DMA is one sequential HBM stream and the FIR's dw-taps are innermost.

The FIR op (FIR3_ANT) is a hand-written 5-uop DVE program with BOTH a 1x
and a 2X_1PORT variant. All its access patterns are flat stride-1 bf16, so
the engine's 2x qualify check passes and it streams the w tensor at 2
elems/lane/cycle: 1.5 cycles per output vs 3 for the 1x segmented-MAC --
the DVE drops off the critical path and the kernel is HBM-bound.

Memory-regime choices (HBM cap ~358 GB/s/NC, 16 SDMA engines):
 - everything device-side is bf16 (host casts f32->bf16, upcasts y back);
   halves the dominant 72MB/core weight traffic; rel err ~4e-3 << 2e-2.
 - the three dh partial planes are combined with two bf16 tensor_adds on the
   DVE (also 2x) -- the DVE has slack now, and this keeps the SDMA engines
   free for the w stream.
 - startup: only the first Rh+2 x rows + the first dh chunk of w tile 0
   gate the first FIR; remaining x tiles load staggered at t=1,3,5.
"""

import sys

sys.path.insert(0, "/opt/trn_rl_repo")

import ml_dtypes
import numpy as np

import concourse.bass as bass
import concourse.bacc as bacc
import concourse.tile as tile
from concourse import mybir
import concourse.bass_isa as bass_isa
from concourse.bass_utils import run_bass_kernel_spmd

import concourse.dve_ops as dve_ops
from concourse.dve_spec import Spec, Src0, Src1
from concourse.dve_uop import (
    DveOpSpec,
    UopConfig,
    UopDpConfig,
    InpSel,
    OutSel,
    OutPath,
    AluOp,
    AluInp,
    DelayInp,
    Trigger,
    ENABLE,
    DISABLE,
)

# ---------------------------------------------------------------------------
# Custom DVE op: 3-tap FIR along the free dim.
#   out[p, j] = sum_dw in0[p, 3j+dw] * in1[p, j+dw]
# in0 = weights interleaved [j][dw] (flat 3N), in1 = x buffer with a 1-elem
# left guard (flat N+2), out = flat N. Hand-written uop programs (escape
# hatch, no Spec lowering): a 3-uop A/B/C loop + 2 prime uops. The 2x
# variant processes an output PAIR per loop lap: per cycle one 32-bit word
# (2 bf16) is fetched per port; x values cross loop laps in stage swap
# flops (refreshed once per lap by the A uop), products hop columns via
# CURR_ALU_OUT (1 col back) and the stage-4 a-flop (2 cols back).
# ---------------------------------------------------------------------------

OP_NAME = "FIR3_ANT"

N, C, H, W = 4, 64, 256, 256
KW = 3
NCORES = 8
HH = H // 2          # rows per core
RB = HH // 2         # rows per partition block (64)
Rh = 8               # rows per h-tile
T = RB // Rh         # h-tiles per core
NXT = 4              # resident x tiles per core
XB = RB // NXT       # local output rows covered per x tile (16)
XR = XB + 2          # rows per resident x tile incl halo
XF = XR * W + 2      # x tile free elems incl 1 zero guard at each end
J = Rh * W           # flattened (h, w) positions per tile
WF = KW * KW * J     # w tile free elems
F32 = mybir.dt.float32
BF16 = mybir.dt.bfloat16
NPBF16 = ml_dtypes.bfloat16

_CACHE = {}


def _fir_ref(in0, in1, c0, c1, c2):
    w = np.asarray(in0, np.float32)
    xb = np.asarray(in1, np.float32)
    P, L = w.shape
    n = L // 3
    w3 = w.reshape(P, n, 3)
    return (
        w3[:, :, 0] * xb[:, 0:n]
        + w3[:, :, 1] * xb[:, 1 : n + 1]
        + w3[:, :, 2] * xb[:, 2 : n + 2]
    )


def _dp(n=8):
    return [UopDpConfig() for _ in range(n)]


def _build_uops_1x():
    """[PRIME1, PRIME2, A, B, C]; sigma2@S0 holds x[j], sigma1@S2 holds
    x[j-1] at A_j entry. Per output j (3 cycles): A consumes w0[j] + one new
    x elem, B consumes w1[j], C consumes w2[j] and writes out[j]."""
    p1 = UopConfig()
    p1.enable_input(InpSel.SRC_1, 2)
    p1.datapath_config = _dp()
    p1.datapath_config[0].pass_through_delay(1)
    p1.datapath_config[1].pass_through_delay(1)
    b = p1.datapath_config[2]
    b.enable_alu(AluOp.BYPASS, AluInp.PREV_DELAY_1, AluInp.PREV_DELAY_1)
    b.swap_enable = ENABLE
    p1.require_inp0, p1.require_inp1 = DISABLE, ENABLE
    p1.repeat_count = 1
    p1.trigger = (Trigger.COUNT, Trigger.NONE, Trigger.NONE)
    p1.next_uop = (1, 0, 0)

    p2 = UopConfig()
    p2.enable_input(InpSel.SRC_1, 1)
    p2.datapath_config = _dp()
    b = p2.datapath_config[0]
    b.enable_alu(AluOp.BYPASS, AluInp.PREV_DELAY_0, AluInp.PREV_DELAY_0)
    b.swap_enable = ENABLE
    p2.require_inp0, p2.require_inp1 = DISABLE, ENABLE
    p2.repeat_count = 1
    p2.trigger = (Trigger.COUNT, Trigger.NONE, Trigger.NONE)
    p2.next_uop = (2, 0, 0)

    ua = UopConfig()
    ua.enable_input(InpSel.SRC_0, 1)
    ua.enable_input(InpSel.SRC_1, 2)
    ua.datapath_config = _dp()
    b = ua.datapath_config[0]
    b.enable_alu(AluOp.BYPASS, AluInp.CURR_SWAP_OUT, AluInp.PREV_DELAY_1)
    b.swap_enable = ENABLE
    b.pass_through_delay(0)
    b = ua.datapath_config[1]
    b.enable_alu(AluOp.BYPASS, AluInp.PREV_ALU_OUT, AluInp.PREV_ALU_OUT)
    b.pass_through_delay(0)
    b = ua.datapath_config[2]
    b.enable_alu(AluOp.BYPASS, AluInp.CURR_SWAP_OUT, AluInp.PREV_ALU_OUT)
    b.swap_enable = ENABLE
    b.pass_through_delay(0)
    b = ua.datapath_config[3]
    b.enable_alu(AluOp.MULTIPLY, AluInp.PREV_ALU_OUT, AluInp.PREV_DELAY_0)
    b = ua.datapath_config[4]
    b.enable_alu(AluOp.BYPASS, AluInp.PREV_ALU_OUT, AluInp.PREV_ALU_OUT)
    b.alu_out_a_enable = ENABLE
    ua.accum_enabled = ENABLE  # builder lint only
    ua.require_inp0, ua.require_inp1 = ENABLE, ENABLE
    ua.repeat_count = 1
    ua.trigger = (Trigger.SRC_TENSOR_DONE, Trigger.COUNT, Trigger.NONE)
    ua.next_uop = (0, 3, 0)

    ub = UopConfig()
    ub.enable_input(InpSel.SRC_0, 1)
    ub.datapath_config = _dp()
    ub.datapath_config[0].pass_through_delay(0)
    ub.datapath_config[1].pass_through_delay(0)
    b = ub.datapath_config[2]
    b.enable_alu(AluOp.MULTIPLY, AluInp.CURR_SWAP_OUT, AluInp.PREV_DELAY_0)
    b = ub.datapath_config[3]
    b.enable_alu(AluOp.BYPASS, AluInp.PREV_ALU_OUT, AluInp.PREV_ALU_OUT)
    ub.require_inp0, ub.require_inp1 = ENABLE, DISABLE
    ub.repeat_count = 1
    ub.trigger = (Trigger.SRC_TENSOR_DONE, Trigger.COUNT, Trigger.NONE)
    ub.next_uop = (0, 4, 0)

    uc = UopConfig()
    uc.enable_input(InpSel.SRC_0, 0)
    uc.datapath_config = _dp()
    b = uc.datapath_config[0]
    b.enable_alu(AluOp.MULTIPLY, AluInp.PREV_ALU_OUT, AluInp.CURR_SWAP_OUT)
    b = uc.datapath_config[1]
    b.enable_alu(AluOp.BYPASS, AluInp.PREV_ALU_OUT, AluInp.PREV_ALU_OUT)
    b = uc.datapath_config[2]
    b.enable_alu(AluOp.BYPASS, AluInp.PREV_ALU_OUT, AluInp.PREV_ALU_OUT)
    b = uc.datapath_config[3]
    b.enable_alu(AluOp.BYPASS, AluInp.PREV_ALU_OUT, AluInp.PREV_ALU_OUT)
    b.enable_delay_from_src(DelayInp.CURR_ALU_OUT, 3)
    b.enable_delay_from_src(DelayInp.NEXT_ALU_OUT_A, 4)
    b = uc.datapath_config[4]
    b.enable_alu(AluOp.ADD, AluInp.PREV_ALU_OUT, AluInp.PREV_DELAY_3)
    b.pass_through_delay(4)
    b = uc.datapath_config[5]
    b.enable_alu(AluOp.ADD, AluInp.PREV_ALU_OUT, AluInp.PREV_DELAY_4)
    b = uc.datapath_config[6]
    b.enable_alu(AluOp.BYPASS, AluInp.PREV_ALU_OUT, AluInp.PREV_ALU_OUT)
    b = uc.datapath_config[7]
    b.enable_alu(AluOp.BYPASS, AluInp.PREV_ALU_OUT, AluInp.PREV_ALU_OUT)
    uc.enable_output(OutSel.ALU_OUT, OutPath.WR0_LO)
    uc.require_inp0, uc.require_inp1 = ENABLE, DISABLE
    uc.repeat_count = 1
    uc.trigger = (Trigger.SRC_TENSOR_DONE, Trigger.COUNT, Trigger.NONE)
    uc.next_uop = (0, 2, 0)

    return [p1, p2, ua, ub, uc]


def _build_uops_2x():
    """[PRIME1(bubble), PRIME2, A, B, C]; per output pair (2i, 2i+1):
    A consumes w-word (w0[2i], w1[2i]) + x-word (x[2i+1], x[2i+2]);
    B consumes (w2[2i], w0[2i+1]); C consumes (w1[2i+1], w2[2i+1]) and
    writes the out pair. sigmaL@S0 = V.lo, sigmaH@S2 = V.hi where V is
    the last consumed x word."""
    p1 = UopConfig()
    p1.datapath_config = _dp()
    p1.require_inp0, p1.require_inp1 = DISABLE, DISABLE
    p1.repeat_count = 1
    p1.trigger = (Trigger.COUNT, Trigger.NONE, Trigger.NONE)
    p1.next_uop = (1, 0, 0)

    p2 = UopConfig()
    p2.enable_input(InpSel.SRC_1, 1)
    p2.enable_input(InpSel.SRC_1_HI, 2)
    p2.datapath_config = _dp()
    b = p2.datapath_config[0]
    b.enable_alu(AluOp.BYPASS, AluInp.PREV_DELAY_0, AluInp.PREV_DELAY_0)
    b.swap_enable = ENABLE
    b.pass_through_delay(1)
    p2.datapath_config[1].pass_through_delay(1)
    b = p2.datapath_config[2]
    b.enable_alu(AluOp.BYPASS, AluInp.PREV_DELAY_1, AluInp.PREV_DELAY_1)
    b.swap_enable = ENABLE
    p2.require_inp0, p2.require_inp1 = DISABLE, ENABLE
    p2.repeat_count = 1
    p2.trigger = (Trigger.COUNT, Trigger.NONE, Trigger.NONE)
    p2.next_uop = (2, 0, 0)

    ua = UopConfig()
    ua.enable_input(InpSel.SRC_0, 1)
    ua.enable_input(InpSel.SRC_0_HI, 2)
    ua.enable_input(InpSel.SRC_1, 3)
    ua.enable_input(InpSel.SRC_1_HI, 4)
    ua.datapath_config = _dp()
    b = ua.datapath_config[0]
    b.enable_alu(AluOp.BYPASS, AluInp.CURR_SWAP_OUT, AluInp.PREV_DELAY_2)
    b.swap_enable = ENABLE
    b.pass_through_delay(0, 1, 3)
    b = ua.datapath_config[1]
    b.enable_alu(AluOp.MULTIPLY, AluInp.PREV_ALU_OUT, AluInp.PREV_DELAY_0)
    b.pass_through_delay(1, 3)
    b = ua.datapath_config[2]
    b.enable_alu(AluOp.BYPASS, AluInp.CURR_SWAP_OUT, AluInp.PREV_DELAY_3)
    b.swap_enable = ENABLE
    b.pass_through_delay(1)
    b.enable_delay_from_src(DelayInp.PREV_ALU_OUT, 0)
    b = ua.datapath_config[3]
    b.enable_alu(AluOp.MULTIPLY, AluInp.PREV_ALU_OUT, AluInp.PREV_DELAY_1)
    b.pass_through_delay(0)
    b = ua.datapath_config[4]
    b.enable_alu(AluOp.ADD, AluInp.PREV_ALU_OUT, AluInp.PREV_DELAY_0)
    b.alu_out_a_enable = ENABLE
    ua.accum_enabled = ENABLE  # builder lint only
    ua.require_inp0, ua.require_inp1 = ENABLE, ENABLE
    ua.repeat_count = 1
    ua.trigger = (Trigger.SRC_TENSOR_DONE, Trigger.COUNT, Trigger.NONE)
    ua.next_uop = (0, 3, 0)

    ub = UopConfig()
    ub.enable_input(InpSel.SRC_0, 0)
    ub.enable_input(InpSel.SRC_0_HI, 1)
    ub.datapath_config = _dp()
    b = ub.datapath_config[0]
    b.enable_alu(AluOp.MULTIPLY, AluInp.PREV_ALU_OUT, AluInp.CURR_SWAP_OUT)
    b.pass_through_delay(0)
    b = ub.datapath_config[1]
    b.enable_alu(AluOp.BYPASS, AluInp.PREV_ALU_OUT, AluInp.PREV_ALU_OUT)
    b.pass_through_delay(0)
    b = ub.datapath_config[2]
    b.enable_alu(AluOp.MULTIPLY, AluInp.CURR_ALU_OUT, AluInp.PREV_DELAY_0)
    b = ub.datapath_config[3]
    b.enable_alu(AluOp.BYPASS, AluInp.PREV_ALU_OUT, AluInp.PREV_ALU_OUT)
    ub.require_inp0, ub.require_inp1 = ENABLE, DISABLE
    ub.repeat_count = 1
    ub.trigger = (Trigger.SRC_TENSOR_DONE, Trigger.COUNT, Trigger.NONE)
    ub.next_uop = (0, 4, 0)

    uc = UopConfig()
    uc.enable_input(InpSel.SRC_0, 0)
    uc.enable_input(InpSel.SRC_0_HI, 1)
    uc.datapath_config = _dp()
    b = uc.datapath_config[0]
    b.enable_alu(AluOp.MULTIPLY, AluInp.PREV_ALU_OUT, AluInp.CURR_SWAP_OUT)
    b.pass_through_delay(0)
    b = uc.datapath_config[1]
    b.enable_alu(AluOp.BYPASS, AluInp.PREV_ALU_OUT, AluInp.PREV_ALU_OUT)
    b.pass_through_delay(0)
    b.enable_delay_from_src(DelayInp.CURR_ALU_OUT, 1)
    b = uc.datapath_config[2]
    b.enable_alu(AluOp.MULTIPLY, AluInp.CURR_SWAP_OUT, AluInp.PREV_DELAY_0)
    b.pass_through_delay(1)
    b.enable_delay_from_src(DelayInp.PREV_ALU_OUT, 2)
    b = uc.datapath_config[3]
    b.enable_alu(AluOp.ADD, AluInp.PREV_ALU_OUT, AluInp.PREV_DELAY_2)
    b.pass_through_delay(1)
    b.enable_delay_from_src(DelayInp.CURR_ALU_OUT, 3)
    b.enable_delay_from_src(DelayInp.NEXT_ALU_OUT_A, 4)
    b = uc.datapath_config[4]
    b.enable_alu(AluOp.ADD, AluInp.PREV_ALU_OUT, AluInp.PREV_DELAY_3)
    b.pass_through_delay(1, 4)
    b = uc.datapath_config[5]
    b.enable_alu(AluOp.ADD, AluInp.PREV_DELAY_4, AluInp.PREV_DELAY_1)
    b.enable_delay_from_src(DelayInp.PREV_ALU_OUT, 2)
    b = uc.datapath_config[6]
    b.enable_alu(AluOp.BYPASS, AluInp.PREV_ALU_OUT, AluInp.PREV_ALU_OUT)
    b.pass_through_delay(2)
    b = uc.datapath_config[7]
    b.enable_alu(AluOp.BYPASS, AluInp.PREV_ALU_OUT, AluInp.PREV_ALU_OUT)
    b.pass_through_delay(2)
    uc.enable_output(OutSel.ALU_OUT, OutPath.WR0_LO)
    uc.enable_output(OutSel.DELAY_2, OutPath.WR0_HI)
    uc.require_inp0, uc.require_inp1 = ENABLE, DISABLE
    uc.repeat_count = 1
    uc.trigger = (Trigger.SRC_TENSOR_DONE, Trigger.COUNT, Trigger.NONE)
    uc.next_uop = (0, 2, 0)

    return [p1, p2, ua, ub, uc]


class FirOp:
    """Duck-typed DveOp with hand-written uop programs."""

    name = OP_NAME
    spec = Spec(body=Src0 * Src1, reference=_fir_ref)
    subdim = False
    uops_sha = {}
    perf_en = {}

    def __init__(self):
        self._cache = {}

    def compile(self, ver):
        if ver in self._cache:
            return self._cache[ver]
        s = DveOpSpec(
            name=self.name,
            opcode=dve_ops.get_dve_sub_opcode(self.name),
            uops=_build_uops_1x(),
            uops_2x=_build_uops_2x(),
            rd1_en=True,
            perf_max=1,
        )
        self._cache[ver] = s
        return s


def get_fir_op():
    existing = getattr(dve_ops, "_ANT_FIR3", None)
    if existing is not None:
        return existing
    op = FirOp()
    dve_ops.OPS.append(op)
    dve_ops._SUB_OPCODE_FOR_NAME[OP_NAME] = (
        dve_ops._CUSTOM_DVE_ROW_BASE + len(dve_ops.OPS) - 1
    )
    assert dve_ops._SUB_OPCODE_FOR_NAME[OP_NAME] < 0x20
    dve_ops.CUSTOM_DVE_SPECS[OP_NAME] = op.spec
    dve_ops._ANT_FIR3 = op
    return op


def emit_fir(vec, op, *, out, in0, in1):
    """_custom_dve clone with perf_max=1 (TTSS shape, no scalars)."""
    nc_bass = vec.bass
    if op.name not in nc_bass.m.ant_custom_dve_ops:
        nc_bass.m.ant_custom_dve_ops = sorted(
            {*nc_bass.m.ant_custom_dve_ops, op.name}
        )
    isa_opcode = nc_bass.isa.Opcode["NEURON_ISA_TPB_OPCODE_CUSTOM_DVE_ANT_0"].value
    zero = mybir.ImmediateValue(dtype=mybir.dt.float32, value=0.0)
    ins = [
        vec.lower_ap(in0, for_isa=True, opt=True),
        vec.lower_ap(in1, for_isa=True, opt=True),
        zero,
        zero,
    ]
    outs = [vec.lower_ap(out, for_isa=True, opt=True)]
    return vec.add_instruction(
        bass_isa.InstCustomDveAnt(
            name=nc_bass.get_next_instruction_name(),
            op_name=op.name,
            rd1_en=True,
            subdim=0,
            imm2=0.0,
            shape=bass_isa.CustomDveShape.TTSS,
            row=dve_ops.get_dve_sub_opcode(op.name),
            isa_opcode=isa_opcode,
            ins=ins,
            outs=outs,
            perf_max=1,
        )
    )


def _build():
    op = get_fir_op()
    nc = bacc.Bacc("TRN2", target_bir_lowering=False, debug=False, num_devices=NCORES)
    x_in = nc.dram_tensor("x", [NXT, 128, XF], BF16, kind="ExternalInput")
    w_in = nc.dram_tensor("w", [T, 128, WF], BF16, kind="ExternalInput")
    y_out = nc.dram_tensor("y", [T, 128, J], BF16, kind="ExternalOutput")

    with tile.TileContext(nc) as tc:
        with (
            tc.tile_pool(name="xp", bufs=1) as xpool,
            tc.tile_pool(name="wp", bufs=3) as wpool,
            tc.tile_pool(name="op", bufs=3) as opool,
            tc.tile_pool(name="pa", bufs=2) as papool,
            tc.tile_pool(name="pb", bufs=2) as pbpool,
        ):
            # x stays resident: NXT tiles, each covering XB output rows
            # (+2 halo rows) per partition block, loaded once. x0 is issued
            # first (tile-0 compute needs it); x1-3 follow behind w0 so the
            # first FIR isn't queued behind 4.7MB of x.
            xtiles = []
            for s in range(NXT):
                xt = xpool.tile([128, XF], BF16, tag=f"x{s}")
                xtiles.append(xt)
            # only the first Rh+2 rows of x0 gate tile 0's compute
            XC0 = (Rh + 2) * W + 2
            nc.scalar.dma_start(
                out=xtiles[0][:, 0:XC0], in_=x_in[0, :, 0:XC0]
            )

            for t in range(T):
                wt = wpool.tile([128, WF], BF16)
                # every w tile loads as 3 per-dh chunks: 12KB packets run at
                # the same per-engine rate as whole-tile, and subtile deps
                # let each FIR start as soon as its own third lands.
                for dh in range(KW):
                    c0 = dh * KW * J
                    nc.sync.dma_start(
                        out=wt[:, c0:c0 + KW * J],
                        in_=w_in[t, :, c0:c0 + KW * J],
                    )
                if t == 0:
                    # backfill the rest of x0 behind w0's chunks
                    nc.scalar.dma_start(
                        out=xtiles[0][:, XC0:XF], in_=x_in[0, :, XC0:XF]
                    )
                if t in (1, 3, 5):
                    # stagger the remaining resident-x loads so they don't
                    # compete with the early w tiles (x_s first needed at
                    # tile 2s).
                    s = (t + 1) // 2
                    nc.scalar.dma_start(out=xtiles[s][:], in_=x_in[s])

                xt = xtiles[t * Rh // XB]
                rbase = t * Rh - (t * Rh // XB) * XB

                ot = opool.tile([128, J], BF16, name="ot")
                pa = papool.tile([128, J], BF16, name="pa")
                pb = pbpool.tile([128, J], BF16, name="pb")
                # one whole-tile FIR per kernel row dh:
                #   tgt[p, j] = sum_dw w[dh, j, dw] * x[(rbase+dh)*W + j + dw - 1]
                # (x AP offset: the +1 guard shift and -1 dw base cancel)
                for dh, tgt in ((0, ot), (1, pa), (2, pb)):
                    emit_fir(
                        nc.vector,
                        op,
                        out=tgt[:, 0:J],
                        in0=wt[:, dh * KW * J:(dh + 1) * KW * J],
                        in1=xt[:, (rbase + dh) * W:(rbase + dh) * W + J + 2],
                    )
                nc.vector.tensor_add(ot[:], ot[:], pa[:])
                nc.vector.tensor_add(ot[:], ot[:], pb[:])
                nc.scalar.dma_start(out=y_out[t], in_=ot[:])
    nc.compile()
    return nc


def _get_nc():
    if "nc" not in _CACHE:
        _CACHE["nc"] = _build()
    return _CACHE["nc"]


def _pack_core(xh_n: np.ndarray, w5_n: np.ndarray, hf: int):
    """Repack one core's shard into per-tile-contiguous DMA blocks.

    xh_n: [C, H+2, W] H-padded x for batch n; w5_n: [C, 9, H, W].
    Returns x_blocks [NXT, 128, XF], w_blocks [T, 128, WF].
    """
    xc = xh_n[:, hf * HH:hf * HH + HH + 2, :]          # [C, HH+2, W]
    wc = w5_n[:, :, hf * HH:(hf + 1) * HH, :]          # [C, 9, HH, W]

    xb = np.zeros((NXT, 2, C, XR * W + 2), dtype=np.float32)
    for s in range(NXT):
        for hb in range(2):
            r0 = hb * RB + s * XB
            xb[s, hb, :, 1:-1] = xc[:, r0:r0 + XR, :].reshape(C, XR * W)
    # w: [C, (dh, dw), (hb, t, r), wd] -> [t, (hb, c), dh, (r, wd), dw]
    wb = (
        wc.reshape(C, KW, KW, 2, T, Rh, W)
        .transpose(4, 3, 0, 1, 5, 6, 2)
        .copy()
    )  # [T, hb, C, dh, r, wd, dw]
    # width-edge taps multiply zero padding in the reference -> zero them
    wb[:, :, :, :, :, 0, 0] = 0.0
    wb[:, :, :, :, :, W - 1, KW - 1] = 0.0
    return (
        xb.reshape(NXT, 128, XF).astype(NPBF16),
        np.ascontiguousarray(wb.reshape(T, 128, WF)).astype(NPBF16),
    )


def _make_in_maps(x: np.ndarray, conv_weights: np.ndarray):
    x = np.asarray(x, dtype=np.float32)
    w5 = np.asarray(conv_weights, dtype=np.float32).reshape(N, C, KW * KW, H, W)
    xh = np.pad(x, ((0, 0), (0, 0), (1, 1), (0, 0)))

    in_maps = []
    for i in range(NCORES):
        n, hf = divmod(i, 2)
        xb, wb = _pack_core(xh[n], w5[n], hf)
        in_maps.append({"x": xb, "w": wb})
    return in_maps


def kernel(x: np.ndarray, conv_weights: np.ndarray) -> np.ndarray:
    nc = _get_nc()
    in_maps = _make_in_maps(x, conv_weights)
    res = run_bass_kernel_spmd(nc, in_maps, list(range(NCORES)))
    out = np.empty((N, C, H, W), dtype=np.float32)
    for i in range(NCORES):
        n, hf = divmod(i, 2)
        yb = res.results[i]["y"].astype(np.float32).reshape(T, 2, C, Rh, W)
        # invert: out rows h = hf*HH + hb*RB + t*Rh + h_sub
        oc = yb.transpose(2, 1, 0, 3, 4).reshape(C, HH, W)
        out[n, :, hf * HH:(hf + 1) * HH, :] = oc
    return out
